# revision 24
# baseline (speedup 1.0000x reference)
"""Trainium2 Bass kernel for nn_CLIPVisionTower_Nuwa_Next (topk_masking).

kernel(**inputs) takes the FULL inputs
    hidden_states_for_aggregation [64, 576, 1024] f32
    hidden_states_for_sim         [64, 576, 1024] f32
    cls_attention_map             [64, 576]       f32
and returns (aggregated [64, 144, 1024] f32, benchmark_indices [64, 144] i32).

Pure data parallel across 8 NeuronCores: 8 images per core. The whole
per-image computation (local 2x2 top-2, global top-K selection, quantile
threshold, penalty-masked similarity aggregation) runs on-device.
"""

import sys
from contextlib import ExitStack

import numpy as np

sys.path.insert(0, "/opt/trn_rl_repo")

import concourse.bass as bass  # noqa: E402
import concourse.tile as tile  # noqa: E402
from concourse import bacc, bass_utils, masks, mybir  # noqa: E402
from concourse._compat import with_exitstack  # noqa: E402

dt = mybir.dt
Alu = mybir.AluOpType
Act = mybir.ActivationFunctionType
AX = mybir.AxisListType

B = 64
IMGS = 8         # images per core
NCORES = 8
N = 576          # tokens
D = 1024         # hidden
K = 144          # kept tokens
NCH = 5          # 576 = 4*128 + 64 partition tiles
PT = [128, 128, 128, 128, 64]
DCH = 8          # 1024 / 128 d-chunks
KT = [128, 16]   # 144 = 128 + 16 k tiles

# "f32" = exact fp32 matmuls (4 cyc/row); "f32r" = reduced-precision fp32
# multiply path (1 cyc/row at moving dim >= 256)
MM_MODE = "f32r"


MMDT = dt.float32r if MM_MODE == "f32r" else dt.float32


def _mm(ap):
    return ap


def host_constants():
    ys, xs = np.meshgrid(np.arange(24), np.arange(24), indexing="ij")
    coords = np.stack([ys, xs], -1).reshape(-1, 2).astype(np.float32)
    dist = np.sqrt(((coords[:, None, :] - coords[None, :, :]) ** 2).sum(-1))
    pen = (1.0 - np.minimum(dist / np.float32(np.sqrt(280.0)), 1.0)).astype(np.float32)
    pen16 = np.zeros((N, 640), np.float16)
    pen16[:, :N] = pen.astype(np.float16)
    iota576 = np.broadcast_to(np.arange(N, dtype=np.float32)[None, :], (128, N)).copy()
    iota144 = np.broadcast_to(np.arange(K, dtype=np.float32)[None, :], (128, K)).copy()
    targets = np.broadcast_to(
        np.array([143.0, 65.0, 64.0], np.float32)[None, :], (128, 3)
    ).copy()
    tcol16 = (np.arange(128)[:, None] + 128 * np.arange(5)[None, :]).astype(np.float16)
    return {
        "pen16": pen16,
        "iota576": iota576,
        "iota144": iota144,
        "targets": targets,
        "tcol16": tcol16,
    }


@with_exitstack
def kern(ctx: ExitStack, tc: tile.TileContext, outs, ins):
    nc = tc.nc
    agg_out, idx_out = outs
    hs_agg, hs_sim, metric, pen16, iota576, iota144, targets, tcol16 = ins

    # ---------------- pools ----------------
    cpool = ctx.enter_context(tc.tile_pool(name="consts", bufs=1))
    spool = ctx.enter_context(tc.tile_pool(name="stage_s", bufs=1))
    sm2 = ctx.enter_context(tc.tile_pool(name="stage_s2", bufs=2))
    hsimp = ctx.enter_context(tc.tile_pool(name="hsim", bufs=7))
    haggp = ctx.enter_context(tc.tile_pool(name="hagg", bufs=7))
    hntp = ctx.enter_context(tc.tile_pool(name="hnt", bufs=16))
    seltp = ctx.enter_context(tc.tile_pool(name="selt", bufs=16))
    hselp = ctx.enter_context(tc.tile_pool(name="hsel", bufs=2))
    penp = ctx.enter_context(tc.tile_pool(name="pen", bufs=2))
    wp = ctx.enter_context(tc.tile_pool(name="w", bufs=2))
    wtp = ctx.enter_context(tc.tile_pool(name="wt", bufs=10))
    sqp = ctx.enter_context(tc.tile_pool(name="sq", bufs=1))
    stgp = ctx.enter_context(tc.tile_pool(name="stg", bufs=3))
    msc = ctx.enter_context(tc.tile_pool(name="misc", bufs=2))
    mscsm = ctx.enter_context(tc.tile_pool(name="misc_small", bufs=2))
    eqp = ctx.enter_context(tc.tile_pool(name="eqp", bufs=5))
    perim = ctx.enter_context(tc.tile_pool(name="per_img", bufs=8))

    pst = ctx.enter_context(tc.tile_pool(name="ps_tp", bufs=2, space="PSUM"))
    pssim = ctx.enter_context(tc.tile_pool(name="ps_sim", bufs=3, space="PSUM"))
    psagg = ctx.enter_context(tc.tile_pool(name="ps_agg", bufs=2, space="PSUM"))
    pssmall = ctx.enter_context(tc.tile_pool(name="ps_small", bufs=1, space="PSUM"))

    # ---------------- constants in SBUF ----------------
    ident = cpool.tile([128, 128], dt.float32)
    masks.make_identity(nc, ident[:])
    io576 = cpool.tile([128, N], dt.float32)
    nc.sync.dma_start(io576[:], iota576[:])
    io144 = cpool.tile([128, K], dt.float32)
    nc.sync.dma_start(io144[:], iota144[:])
    tgt = cpool.tile([128, 3], dt.float32)
    nc.sync.dma_start(tgt[:], targets[:])
    tcol = cpool.tile([128, 5], dt.float16)
    nc.sync.dma_start(tcol[:], tcol16[:])
    ones = cpool.tile([128, N], dt.float32)
    nc.gpsimd.memset(ones[:], 1.0)
    neg = cpool.tile([IMGS, N], dt.float32)
    nc.gpsimd.memset(neg[:], -1e30)

    # ---------------- stage S: selection logic (all 8 images) ----------------
    M = spool.tile([IMGS, N], dt.float32)
    nc.sync.dma_start(M[:], metric[:])

    # region (2x2) top-2 via a min/max tournament (all views <= 3 free dims).
    # token = 48*ry + 24*dy + 2*c + dx; g = 2*ry + dy
    cs = spool.tile([IMGS, 288], dt.float32)       # [m1 | s2] candidate values
    mg = M[:].rearrange("p (g c dx) -> p g c dx", g=24, c=12, dx=2)
    mxd = sm2.tile([IMGS, 288], dt.float32, tag="scr")   # row max over dx
    mnd = sm2.tile([IMGS, 288], dt.float32, tag="scr")   # row min over dx
    nc.vector.tensor_tensor(
        mxd[:].rearrange("p (g c) -> p g c", g=24, c=12),
        mg[:, :, :, 0], mg[:, :, :, 1], Alu.max,
    )
    nc.vector.tensor_tensor(
        mnd[:].rearrange("p (g c) -> p g c", g=24, c=12),
        mg[:, :, :, 0], mg[:, :, :, 1], Alu.min,
    )

    def dyv(t, d):
        return t[:].rearrange("p (ry dy c) -> p ry dy c", ry=12, dy=2, c=12)[:, :, d, :]

    cs12 = cs[:, 0:144].rearrange("p (ry c) -> p ry c", ry=12, c=12)
    nc.vector.tensor_tensor(cs12, dyv(mxd, 0), dyv(mxd, 1), Alu.max)  # m1
    tmn = sm2.tile([IMGS, 144], dt.float32, tag="tmn")
    tmn12 = tmn[:].rearrange("p (ry c) -> p ry c", ry=12, c=12)
    nc.vector.tensor_tensor(tmn12, dyv(mxd, 0), dyv(mxd, 1), Alu.min)
    tmx = sm2.tile([IMGS, 144], dt.float32, tag="tmx")
    tmx12 = tmx[:].rearrange("p (ry c) -> p ry c", ry=12, c=12)
    nc.vector.tensor_tensor(tmx12, dyv(mnd, 0), dyv(mnd, 1), Alu.max)
    nc.vector.tensor_tensor(cs[:, 144:288], tmn[:], tmx[:], Alu.max)  # s2

    # expand s2 to token order (4 strided copies), then candidate mask + ms
    s2x = sm2.tile([IMGS, N], dt.float32, tag="scr")
    s2src = cs[:, 144:288].rearrange("p (ry c) -> p ry c", ry=12, c=12)
    s2xv = s2x[:].rearrange("p (ry dy c dx) -> p ry dy c dx", ry=12, dy=2, c=12, dx=2)
    for dv in range(2):
        for dx in range(2):
            nc.vector.tensor_copy(s2xv[:, :, dv, :, dx], s2src)
    CAND = sm2.tile([IMGS, N], dt.uint8, tag="scr8")
    nc.vector.tensor_tensor(CAND[:], M[:], s2x[:], Alu.is_ge)
    MS = spool.tile([IMGS, N], dt.float32)
    nc.vector.tensor_copy(MS[:], neg[:])
    nc.vector.copy_predicated(MS[:], CAND[:], M[:])

    # csT columns
    CST = []
    for t in range(3):
        pt = 128 if t < 2 else 32
        ps = pst.tile([128, 512], dt.float32, tag="tp")
        nc.tensor.transpose(ps[0:pt, 0:IMGS], cs[:, t * 128 : t * 128 + pt], ident[0:IMGS, 0:IMGS])
        sb = spool.tile([128, IMGS], dt.float32, tag=f"cst{t}")
        nc.scalar.copy(sb[0:pt, :], ps[0:pt, 0:IMGS])
        CST.append(sb)

    # ranks + (theta, s78, s79) extraction
    EXTR = spool.tile([1, 3 * IMGS], dt.float32)
    for img in range(IMGS):
        CS0 = sm2.tile([1, 288], dt.float32, tag="cs0")
        nc.sync.dma_start(CS0[:], cs[img : img + 1, :])
        BC = sm2.tile([128, 288], dt.float32, tag="bc")
        nc.gpsimd.partition_broadcast(BC[:], CS0[:])
        RK = sm2.tile([128, 3], dt.float32, tag="rk")
        RS = sm2.tile([128, 288], dt.float32, tag="rs")
        psx = pssmall.tile([2, K], dt.float32, tag="small")
        for t in range(3):
            pt = 128 if t < 2 else 32
            nc.vector.tensor_scalar(
                RS[0:pt, :], BC[0:pt, :], CST[t][0:pt, img : img + 1], None,
                op0=Alu.is_gt, op1=Alu.add, accum_out=RK[0:pt, t : t + 1],
            )
            IND = sm2.tile([128, 3], dt.float32, tag="ind")
            nc.vector.tensor_scalar(
                IND[0:pt, :], tgt[0:pt, :], RK[0:pt, t : t + 1], None, op0=Alu.is_equal
            )
            nc.tensor.matmul(
                psx[0:1, 0:3], CST[t][0:pt, img : img + 1], IND[0:pt, :],
                start=(t == 0), stop=(t == 2),
            )
        nc.vector.tensor_copy(EXTR[0:1, 3 * img : 3 * img + 3], psx[0:1, 0:3])

    # redistribute to [8, 3]; thresholds
    EXT = spool.tile([IMGS, 3], dt.float32)
    nc.sync.dma_start(EXT[:, :], EXTR[0:1, :])
    TH = spool.tile([IMGS, 1], dt.float32)   # theta - delta
    nc.vector.tensor_scalar(TH[:], EXT[:, 0:1], -1e-5, None, op0=Alu.add)
    T1 = spool.tile([IMGS, 1], dt.float32)
    nc.vector.tensor_scalar(T1[:], EXT[:, 2:3], 0.65, None, op0=Alu.mult)
    THR = spool.tile([IMGS, 1], dt.float32)
    nc.vector.scalar_tensor_tensor(THR[:], EXT[:, 1:2], 0.35, T1[:], op0=Alu.mult, op1=Alu.add)

    SEL = spool.tile([IMGS, N], dt.float32)
    nc.vector.tensor_scalar(SEL[:], MS[:], TH[:], None, op0=Alu.is_ge)
    HI = spool.tile([IMGS, N], dt.float32)
    nc.vector.tensor_scalar(HI[:], MS[:], THR[:], None, op0=Alu.is_ge)

    PS = sm2.tile([IMGS, N], dt.float32, tag="scr")
    nc.vector.tensor_tensor_scan(PS[:], SEL[:], SEL[:], 0.0, op0=Alu.add, op1=Alu.bypass)
    SPOS = spool.tile([IMGS, N], dt.float32)
    nc.vector.scalar_tensor_tensor(SPOS[:], PS[:], 0.0, SEL[:], op0=Alu.bypass, op1=Alu.mult)
    nc.vector.tensor_scalar(SPOS[:], SPOS[:], -1.0, None, op0=Alu.add)

    SPT, HIT = [], []
    for t in range(NCH):
        pt = PT[t]
        ps = pst.tile([128, 512], dt.float32, tag="tp")
        nc.tensor.transpose(ps[0:pt, 0:IMGS], SPOS[:, t * 128 : t * 128 + pt], ident[0:IMGS, 0:IMGS])
        sb = spool.tile([128, IMGS], dt.float32, tag=f"spt{t}")
        nc.scalar.copy(sb[0:pt, :], ps[0:pt, 0:IMGS])
        SPT.append(sb)
        ps2 = pst.tile([128, 512], dt.float32, tag="tp")
        nc.tensor.transpose(ps2[0:pt, 0:IMGS], HI[:, t * 128 : t * 128 + pt], ident[0:IMGS, 0:IMGS])
        sb2 = spool.tile([128, IMGS], dt.float16, tag=f"hit{t}")
        nc.scalar.copy(sb2[0:pt, :], ps2[0:pt, 0:IMGS])
        HIT.append(sb2)

    # per image: eq matmuls -> benchmark indices + high flags
    IDX16 = []
    SELC = []
    HIC = []
    for img in range(IMGS):
        ps_ixhi = pssmall.tile([2, K], dt.float32, tag="small")
        EQS5 = []
        for c in range(NCH):
            pc = PT[c]
            EQ = eqp.tile([128, K], dt.float16, tag="eq")
            nc.vector.tensor_scalar(
                EQ[0:pc, :], io144[0:pc, :], SPT[c][0:pc, img : img + 1], None,
                op0=Alu.is_equal,
            )
            LH = mscsm.tile([128, 2], dt.float16, tag="lh")
            nc.scalar.copy(LH[0:pc, 0:1], tcol[0:pc, c : c + 1])
            nc.scalar.copy(LH[0:pc, 1:2], HIT[c][0:pc, img : img + 1])
            nc.tensor.matmul(
                ps_ixhi[:, :], LH[0:pc, :], EQ[0:pc, :],
                start=(c == 0), stop=(c == 4),
            )
            EQS5.append(EQ)
        oi = mscsm.tile([1, K], dt.int32, tag="oi")
        nc.vector.tensor_copy(oi[:, :], ps_ixhi[0:1, :])
        nc.sync.dma_start(idx_out[img : img + 1, :], oi[0:1, :])
        HX = mscsm.tile([2, K], dt.float32, tag="hx")
        nc.scalar.copy(HX[:, :], ps_ixhi[0:2, :])
        psb = pst.tile([128, 512], dt.float32, tag="tp")
        nc.tensor.transpose(psb[0:128, 0:2], HX[0:2, 0:128], ident[0:2, 0:2])
        nc.tensor.transpose(psb[0:16, 2:4], HX[0:2, 128:144], ident[0:2, 0:2])
        sc = perim.tile([128, 2], dt.float32, tag="selc")
        nc.scalar.copy(sc[0:128, 0:1], psb[0:128, 0:1])
        nc.scalar.copy(sc[0:16, 1:2], psb[0:16, 2:3])
        hc = perim.tile([128, 2], dt.float32, tag="hic")
        nc.vector.tensor_scalar(hc[0:128, 0:1], psb[0:128, 1:2], -1.0, 1.0, op0=Alu.mult, op1=Alu.add)
        nc.vector.tensor_scalar(hc[0:16, 1:2], psb[0:16, 3:4], -1.0, 1.0, op0=Alu.mult, op1=Alu.add)
        SELC.append(sc)
        HIC.append(hc)
        # second chain: indices in 16-wrap order for dma_gather idx tiles
        ps_perm = pssmall.tile([2, K], dt.float32, tag="small")
        for c in range(NCH):
            pc = PT[c]
            eqpi = EQS5[c][0:pc, :].rearrange("p (s q) -> p q s", s=9, q=16)
            nc.tensor.matmul(
                ps_perm[0:1, :], tcol[0:pc, c : c + 1], eqpi,
                start=(c == 0), stop=(c == 4),
            )
        i16p = mscsm.tile([1, K], dt.int16, tag="i16p")
        nc.vector.tensor_copy(i16p[:, :], ps_perm[0:1, :])
        IDXS = perim.tile([128, 9], dt.int16, tag="idxs")
        for b in range(8):
            nc.sync.dma_start(IDXS[16 * b : 16 * b + 16, :], i16p[0:1, :])
        IDX16.append(IDXS)

    # ---------------- stage H: heavy per-image pipeline ----------------
    for img in range(IMGS):
        hsim = []
        for t in range(NCH):
            pt = PT[t]
            h = hsimp.tile([128, D], dt.float32, tag="hsim")
            nc.sync.dma_start(h[0:pt, :], hs_sim[img, t * 128 : t * 128 + pt, :])
            hsim.append(h)
        ssq = mscsm.tile([128, 5], dt.float32, tag="ssq")
        nc.gpsimd.memset(ssq[:, :], 1.0)
        for t in range(NCH):
            pt = PT[t]
            sq = sqp.tile([128, D], dt.float32, tag="sq")
            nc.scalar.activation(
                sq[0:pt, :], hsim[t][0:pt, :], Act.Square, accum_out=ssq[0:pt, t : t + 1]
            )
        nrm = mscsm.tile([128, 5], dt.float32, tag="nrm")
        nc.scalar.sqrt(nrm[:, :], ssq[:, :])
        inv = mscsm.tile([128, 5], dt.float32, tag="inv")
        nc.vector.reciprocal(inv[:, :], nrm[:, :])
        for t in range(NCH):
            pt = PT[t]
            nc.vector.tensor_scalar_mul(hsim[t][0:pt, :], hsim[t][0:pt, :], inv[0:pt, t : t + 1])

        # transpose normalized rows -> hnT[db] [128(d), 576(j)]
        hnT = []
        for db in range(DCH):
            ht = hntp.tile([128, N], MMDT, tag="hnt")
            ps4 = pst.tile([128, 512], dt.float32, tag="tp")
            for jb in range(4):
                nc.tensor.transpose(
                    ps4[:, jb * 128 : (jb + 1) * 128],
                    hsim[jb][:, db * 128 : (db + 1) * 128],
                    ident[:, :],
                )
            if db % 2 == 0:
                nc.scalar.copy(ht[:, 0:512], ps4[:, 0:512])
            else:
                nc.vector.tensor_copy(ht[:, 0:512], ps4[:, 0:512])
            ps1 = pst.tile([128, 512], dt.float32, tag="tp")
            nc.tensor.transpose(
                ps1[:, 0:64], hsim[4][0:64, db * 128 : (db + 1) * 128], ident[0:64, 0:64]
            )
            if db % 2 == 0:
                nc.scalar.copy(ht[:, 512:576], ps1[:, 0:64])
            else:
                nc.vector.tensor_copy(ht[:, 512:576], ps1[:, 0:64])
            hnT.append(ht)

        # gather raw selected rows, transpose -> selT[db] [128(d), 144(k)]
        hsel = hselp.tile([128, 2 * D], dt.float32, tag="hsel")
        hselv = hsel[:].rearrange("p (c d) -> p c d", c=2, d=D)
        nc.gpsimd.dma_gather(
            hselv, hs_sim[img].flatten_outer_dims(), IDX16[img][:, :],
            num_idxs=K, num_idxs_reg=K, elem_size=D, queue_num=0,
        )
        selT = []
        for db in range(DCH):
            ps = pst.tile([128, 512], dt.float32, tag="tp")
            nc.tensor.transpose(
                ps[:, 0:128], hsel[:, db * 128 : (db + 1) * 128], ident[:, :]
            )
            nc.tensor.transpose(
                ps[:, 128:144],
                hsel[0:16, D + db * 128 : D + (db + 1) * 128],
                ident[0:16, 0:16],
            )
            st = seltp.tile([128, K], MMDT, tag="selt")
            nc.scalar.copy(st[:, :], ps[:, 0:144])
            selT.append(st)

        # gather penalty rows (fp16, padded table)
        peng = penp.tile([128, 2 * 640], dt.float16, tag="peng")
        pengv = peng[:].rearrange("p (c d) -> p c d", c=2, d=640)
        nc.gpsimd.dma_gather(
            pengv, pen16[:].flatten_outer_dims(), IDX16[img][:, :],
            num_idxs=K, num_idxs_reg=K, elem_size=640, queue_num=0,
        )

        hagg = []
        for t in range(NCH):
            pt = PT[t]
            h = haggp.tile([128, D], MMDT, tag="hagg")
            if MM_MODE == "f32r":
                stg = stgp.tile([128, D], dt.float32, tag="stg")
                nc.sync.dma_start(stg[0:pt, :], hs_agg[img, t * 128 : t * 128 + pt, :])
                if t % 2 == 0:
                    nc.vector.tensor_copy(h[0:pt, :], stg[0:pt, :])
                else:
                    nc.scalar.copy(h[0:pt, :], stg[0:pt, :])
            else:
                nc.sync.dma_start(h[0:pt, :].bitcast(dt.float32), hs_agg[img, t * 128 : t * 128 + pt, :])
            hagg.append(h)

        # sim matmuls + epilogue per k-tile; W^T built for the AGG matmul
        WTs = []
        for _wti in range(NCH):
            wt_t = wtp.tile([128, K], MMDT, tag="wt")
            WTs.append(wt_t)
        for kt in range(2):
            pk = KT[kt]
            pssA = pssim.tile([128, 288], dt.float32, tag="sim")
            pssB = pssim.tile([128, 288], dt.float32, tag="sim")
            for db in range(DCH):
                lhs = selT[db][:, kt * 128 : kt * 128 + pk]
                nc.tensor.matmul(
                    pssA[0:pk, :], _mm(lhs), _mm(hnT[db][:, 0:288]),
                    start=(db == 0), stop=(db == 7),
                )
                nc.tensor.matmul(
                    pssB[0:pk, :], _mm(lhs), _mm(hnT[db][:, 288:576]),
                    start=(db == 0), stop=(db == 7),
                )
            reluS = wp.tile([128, N], dt.float32, tag="relu")
            nc.scalar.activation(reluS[0:pk, 0:288], pssA[0:pk, :], Act.Relu)
            nc.scalar.activation(reluS[0:pk, 288:576], pssB[0:pk, :], Act.Relu)
            cpen = wp.tile([128, N], dt.float32, tag="cpen")
            rsum = mscsm.tile([128, 1], dt.float32, tag="rsum")
            nc.vector.tensor_tensor(
                cpen[0:pk, :], reluS[0:pk, :],
                peng[0:pk, kt * 640 : kt * 640 + 576], Alu.mult,
            )
            nc.vector.tensor_reduce(rsum[0:pk, :], cpen[0:pk, :], axis=AX.X, op=Alu.add)
            rq = mscsm.tile([128, 1], dt.float32, tag="rq")
            nc.vector.tensor_scalar(rq[0:pk, :], rsum[0:pk, :], 1e-8, None, op0=Alu.add)
            nc.vector.reciprocal(rq[0:pk, :], rq[0:pk, :])
            W = wp.tile([128, N], dt.float32, tag="wtile")
            nc.vector.tensor_scalar(
                W[0:pk, :], cpen[0:pk, :], rq[0:pk, :], HIC[img][0:pk, kt : kt + 1],
                op0=Alu.mult, op1=Alu.mult,
            )
            EQS = wp.tile([128, N], dt.uint8, tag="eqs")
            nc.vector.tensor_scalar(
                EQS[0:pk, :], io576[0:pk, :], SELC[img][0:pk, kt : kt + 1], None,
                op0=Alu.is_equal,
            )
            nc.vector.copy_predicated(W[0:pk, :], EQS[0:pk, :], ones[0:pk, :])
            for jb in range(NCH):
                pj = PT[jb]
                psw = pst.tile([128, 512], dt.float32, tag="tp")
                nc.tensor.transpose(
                    psw[0:pj, 0:pk], W[0:pk, jb * 128 : jb * 128 + pj],
                    ident[0:pk, 0:pk],
                )
                nc.scalar.copy(WTs[jb][0:pj, kt * 128 : kt * 128 + pk], psw[0:pj, 0:pk])

        # AGG: out[k, dd] = sum_j WT[j, k] * hagg[j, dd]
        for kt in range(2):
            pk = KT[kt]
            ost = msc.tile([128, D], dt.float32, tag="ost")
            for h in range(2):
                pa = psagg.tile([128, 512], dt.float32, tag="agg")
                for jb in range(NCH):
                    pj = PT[jb]
                    nc.tensor.matmul(
                        pa[0:pk, :],
                        _mm(WTs[jb][0:pj, kt * 128 : kt * 128 + pk]),
                        _mm(hagg[jb][0:pj, h * 512 : (h + 1) * 512]),
                        start=(jb == 0), stop=(jb == 4),
                    )
                if h == 0:
                    nc.scalar.copy(ost[0:pk, 0:512], pa[0:pk, :])
                else:
                    nc.vector.tensor_copy(ost[0:pk, 512:1024], pa[0:pk, :])
            nc.sync.dma_start(
                agg_out[img, kt * 128 : kt * 128 + pk, :], ost[0:pk, :]
            )


_CACHE = {}


def _build():
    if "nc" in _CACHE:
        return _CACHE["nc"], _CACHE["names"]
    nc = bacc.Bacc("TRN2", target_bir_lowering=False, debug=False)
    in_specs = [
        ("hs_agg", [IMGS, N, D], dt.float32),
        ("hs_sim", [IMGS, N, D], dt.float32),
        ("metric", [IMGS, N], dt.float32),
        ("pen16", [N, 640], dt.float16),
        ("iota576", [128, N], dt.float32),
        ("iota144", [128, K], dt.float32),
        ("targets", [128, 3], dt.float32),
        ("tcol16", [128, 5], dt.float16),
    ]
    ins = [nc.dram_tensor(n, s, d, kind="ExternalInput").ap() for n, s, d in in_specs]
    outs = [
        nc.dram_tensor("agg", [IMGS, K, D], dt.float32, kind="ExternalOutput").ap(),
        nc.dram_tensor("idx", [IMGS, K], dt.int32, kind="ExternalOutput").ap(),
    ]
    with tile.TileContext(nc) as tc:
        kern(tc, outs, ins)
    nc.compile()
    _CACHE["nc"] = nc
    _CACHE["names"] = [n for n, _, _ in in_specs]
    return nc, _CACHE["names"]


def _run(inputs, trace=False, trace_kwargs=None):
    nc, names = _build()
    consts = host_constants()
    in_maps = []
    for core in range(NCORES):
        sl = slice(core * IMGS, (core + 1) * IMGS)
        m = {
            "hs_agg": np.ascontiguousarray(
                inputs["hidden_states_for_aggregation"][sl]
            ).astype(np.float32, copy=False),
            "hs_sim": np.ascontiguousarray(
                inputs["hidden_states_for_sim"][sl]
            ).astype(np.float32, copy=False),
            "metric": np.ascontiguousarray(inputs["cls_attention_map"][sl]).astype(
                np.float32, copy=False
            ),
        }
        m.update(consts)
        in_maps.append(m)
    res = bass_utils.run_bass_kernel_spmd(
        nc, in_maps, core_ids=list(range(NCORES)), trace=trace,
        trace_kwargs=trace_kwargs or {},
    )
    agg = np.concatenate([res.results[c]["agg"] for c in range(NCORES)], axis=0)
    idx = np.concatenate([res.results[c]["idx"] for c in range(NCORES)], axis=0)
    return (agg, idx.astype(np.int32)), res


def kernel(**inputs):
    (agg, idx), _ = _run(inputs, trace=False)
    return agg, idx


if __name__ == "__main__":
    rng = np.random.default_rng(0)
    ins = {
        "hidden_states_for_aggregation": rng.standard_normal((B, N, D), dtype=np.float32),
        "hidden_states_for_sim": rng.standard_normal((B, N, D), dtype=np.float32),
        "cls_attention_map": rng.random((B, N), dtype=np.float32),
    }
    out, idx = kernel(**ins)
    print(out.shape, idx.shape, idx.dtype)


# revision 25
# speedup vs baseline: 1.1373x; 1.1373x over previous
"""Trainium2 Bass kernel for nn_CLIPVisionTower_Nuwa_Next (topk_masking).

kernel(**inputs) takes the FULL inputs
    hidden_states_for_aggregation [64, 576, 1024] f32
    hidden_states_for_sim         [64, 576, 1024] f32
    cls_attention_map             [64, 576]       f32
and returns (aggregated [64, 144, 1024] f32, benchmark_indices [64, 144] i32).

Pure data parallel across 8 NeuronCores: 8 images per core. The whole
per-image computation (local 2x2 top-2, global top-K selection, quantile
threshold, penalty-masked similarity aggregation) runs on-device.
"""

import sys
from contextlib import ExitStack

import numpy as np

sys.path.insert(0, "/opt/trn_rl_repo")

import concourse.bass as bass  # noqa: E402
import concourse.tile as tile  # noqa: E402
from concourse import bacc, bass_utils, masks, mybir  # noqa: E402
from concourse._compat import with_exitstack  # noqa: E402

dt = mybir.dt
Alu = mybir.AluOpType
Act = mybir.ActivationFunctionType
AX = mybir.AxisListType

B = 64
IMGS = 8         # images per core
NCORES = 8
N = 576          # tokens
D = 1024         # hidden
K = 144          # kept tokens
NCH = 5          # 576 = 4*128 + 64 partition tiles
PT = [128, 128, 128, 128, 64]
DCH = 8          # 1024 / 128 d-chunks
KT = [128, 16]   # 144 = 128 + 16 k tiles

# "f32" = exact fp32 matmuls (4 cyc/row); "f32r" = reduced-precision fp32
# multiply path (1 cyc/row at moving dim >= 256)
MM_MODE = "f32r"


MMDT = dt.float32r if MM_MODE == "f32r" else dt.float32


def _mm(ap):
    return ap


def host_constants():
    ys, xs = np.meshgrid(np.arange(24), np.arange(24), indexing="ij")
    coords = np.stack([ys, xs], -1).reshape(-1, 2).astype(np.float32)
    dist = np.sqrt(((coords[:, None, :] - coords[None, :, :]) ** 2).sum(-1))
    pen = (1.0 - np.minimum(dist / np.float32(np.sqrt(280.0)), 1.0)).astype(np.float32)
    pen16 = np.zeros((N, 640), np.float16)
    pen16[:, :N] = pen.astype(np.float16)
    iota576 = np.broadcast_to(np.arange(N, dtype=np.float32)[None, :], (128, N)).copy()
    iota144 = np.broadcast_to(np.arange(K, dtype=np.float32)[None, :], (128, K)).copy()
    targets = np.broadcast_to(
        np.array([143.0, 65.0, 64.0], np.float32)[None, :], (128, 3)
    ).copy()
    tcol16 = (np.arange(128)[:, None] + 128 * np.arange(5)[None, :]).astype(np.float16)
    return {
        "pen16": pen16,
        "iota576": iota576,
        "iota144": iota144,
        "targets": targets,
        "tcol16": tcol16,
    }


@with_exitstack
def kern(ctx: ExitStack, tc: tile.TileContext, outs, ins):
    nc = tc.nc
    agg_out, idx_out = outs
    hs_agg, hs_sim, metric, pen16, iota576, iota144, targets, tcol16 = ins

    # ---------------- pools ----------------
    cpool = ctx.enter_context(tc.tile_pool(name="consts", bufs=1))
    spool = ctx.enter_context(tc.tile_pool(name="stage_s", bufs=1))
    sm2 = ctx.enter_context(tc.tile_pool(name="stage_s2", bufs=2))
    hsimp = ctx.enter_context(tc.tile_pool(name="hsim", bufs=7))
    haggp = ctx.enter_context(tc.tile_pool(name="hagg", bufs=7))
    hntp = ctx.enter_context(tc.tile_pool(name="hnt", bufs=16))
    seltp = ctx.enter_context(tc.tile_pool(name="selt", bufs=16))
    hselp = ctx.enter_context(tc.tile_pool(name="hsel", bufs=2))
    penp = ctx.enter_context(tc.tile_pool(name="pen", bufs=2))
    wp = ctx.enter_context(tc.tile_pool(name="w", bufs=2))
    wtp = ctx.enter_context(tc.tile_pool(name="wt", bufs=10))
    sqp = ctx.enter_context(tc.tile_pool(name="sq", bufs=1))
    stgp = ctx.enter_context(tc.tile_pool(name="stg", bufs=3))
    msc = ctx.enter_context(tc.tile_pool(name="misc", bufs=2))
    mscsm = ctx.enter_context(tc.tile_pool(name="misc_small", bufs=2))
    eqp = ctx.enter_context(tc.tile_pool(name="eqp", bufs=5))
    perim = ctx.enter_context(tc.tile_pool(name="per_img", bufs=8))

    pst = ctx.enter_context(tc.tile_pool(name="ps_tp", bufs=3, space="PSUM"))
    pssim = ctx.enter_context(tc.tile_pool(name="ps_sim", bufs=2, space="PSUM"))
    psagg = ctx.enter_context(tc.tile_pool(name="ps_agg", bufs=2, space="PSUM"))
    pssmall = ctx.enter_context(tc.tile_pool(name="ps_small", bufs=1, space="PSUM"))

    # ---------------- constants in SBUF ----------------
    ident = cpool.tile([128, 128], dt.float32)
    masks.make_identity(nc, ident[:])
    io576 = cpool.tile([128, N], dt.float32)
    nc.sync.dma_start(io576[:], iota576[:])
    io144 = cpool.tile([128, K], dt.float32)
    nc.sync.dma_start(io144[:], iota144[:])
    tgt = cpool.tile([128, 3], dt.float32)
    nc.sync.dma_start(tgt[:], targets[:])
    tcol = cpool.tile([128, 5], dt.float16)
    nc.sync.dma_start(tcol[:], tcol16[:])
    ones = cpool.tile([128, N], dt.float32)
    nc.gpsimd.memset(ones[:], 1.0)
    neg = cpool.tile([IMGS, N], dt.float32)
    nc.gpsimd.memset(neg[:], -1e30)

    # ---------------- stage S: selection logic (all 8 images) ----------------
    M = spool.tile([IMGS, N], dt.float32)
    nc.sync.dma_start(M[:], metric[:])

    # region (2x2) top-2 via a min/max tournament (all views <= 3 free dims).
    # token = 48*ry + 24*dy + 2*c + dx; g = 2*ry + dy
    cs = spool.tile([IMGS, 288], dt.float32)       # [m1 | s2] candidate values
    mg = M[:].rearrange("p (g c dx) -> p g c dx", g=24, c=12, dx=2)
    mxd = sm2.tile([IMGS, 288], dt.float32, tag="scr")   # row max over dx
    mnd = sm2.tile([IMGS, 288], dt.float32, tag="scr")   # row min over dx
    nc.vector.tensor_tensor(
        mxd[:].rearrange("p (g c) -> p g c", g=24, c=12),
        mg[:, :, :, 0], mg[:, :, :, 1], Alu.max,
    )
    nc.vector.tensor_tensor(
        mnd[:].rearrange("p (g c) -> p g c", g=24, c=12),
        mg[:, :, :, 0], mg[:, :, :, 1], Alu.min,
    )

    def dyv(t, d):
        return t[:].rearrange("p (ry dy c) -> p ry dy c", ry=12, dy=2, c=12)[:, :, d, :]

    cs12 = cs[:, 0:144].rearrange("p (ry c) -> p ry c", ry=12, c=12)
    nc.vector.tensor_tensor(cs12, dyv(mxd, 0), dyv(mxd, 1), Alu.max)  # m1
    tmn = sm2.tile([IMGS, 144], dt.float32, tag="tmn")
    tmn12 = tmn[:].rearrange("p (ry c) -> p ry c", ry=12, c=12)
    nc.vector.tensor_tensor(tmn12, dyv(mxd, 0), dyv(mxd, 1), Alu.min)
    tmx = sm2.tile([IMGS, 144], dt.float32, tag="tmx")
    tmx12 = tmx[:].rearrange("p (ry c) -> p ry c", ry=12, c=12)
    nc.vector.tensor_tensor(tmx12, dyv(mnd, 0), dyv(mnd, 1), Alu.max)
    nc.vector.tensor_tensor(cs[:, 144:288], tmn[:], tmx[:], Alu.max)  # s2

    # expand s2 to token order (4 strided copies), then candidate mask + ms
    s2x = sm2.tile([IMGS, N], dt.float32, tag="scr")
    s2src = cs[:, 144:288].rearrange("p (ry c) -> p ry c", ry=12, c=12)
    s2xv = s2x[:].rearrange("p (ry dy c dx) -> p ry dy c dx", ry=12, dy=2, c=12, dx=2)
    for dv in range(2):
        for dx in range(2):
            nc.vector.tensor_copy(s2xv[:, :, dv, :, dx], s2src)
    CAND = sm2.tile([IMGS, N], dt.uint8, tag="scr8")
    nc.vector.tensor_tensor(CAND[:], M[:], s2x[:], Alu.is_ge)
    MS = spool.tile([IMGS, N], dt.float32)
    nc.vector.tensor_copy(MS[:], neg[:])
    nc.vector.copy_predicated(MS[:], CAND[:], M[:])

    # csT columns
    CST = []
    for t in range(3):
        pt = 128 if t < 2 else 32
        ps = pst.tile([128, 512], dt.float32, tag="tp")
        nc.tensor.transpose(ps[0:pt, 0:IMGS], cs[:, t * 128 : t * 128 + pt], ident[0:IMGS, 0:IMGS])
        sb = spool.tile([128, IMGS], dt.float32, tag=f"cst{t}")
        nc.scalar.copy(sb[0:pt, :], ps[0:pt, 0:IMGS])
        CST.append(sb)

    # ranks + (theta, s78, s79) extraction
    EXTR = spool.tile([1, 3 * IMGS], dt.float32)
    for img in range(IMGS):
        CS0 = sm2.tile([1, 288], dt.float32, tag="cs0")
        nc.sync.dma_start(CS0[:], cs[img : img + 1, :])
        BC = sm2.tile([128, 288], dt.float32, tag="bc")
        nc.gpsimd.partition_broadcast(BC[:], CS0[:])
        RK = sm2.tile([128, 3], dt.float32, tag="rk")
        RS = sm2.tile([128, 288], dt.float32, tag="rs")
        psx = pssmall.tile([2, K], dt.float32, tag="small")
        for t in range(3):
            pt = 128 if t < 2 else 32
            nc.vector.tensor_scalar(
                RS[0:pt, :], BC[0:pt, :], CST[t][0:pt, img : img + 1], None,
                op0=Alu.is_gt, op1=Alu.add, accum_out=RK[0:pt, t : t + 1],
            )
            IND = sm2.tile([128, 3], dt.float32, tag="ind")
            nc.vector.tensor_scalar(
                IND[0:pt, :], tgt[0:pt, :], RK[0:pt, t : t + 1], None, op0=Alu.is_equal
            )
            nc.tensor.matmul(
                psx[0:1, 0:3], CST[t][0:pt, img : img + 1], IND[0:pt, :],
                start=(t == 0), stop=(t == 2),
            )
        nc.vector.tensor_copy(EXTR[0:1, 3 * img : 3 * img + 3], psx[0:1, 0:3])

    # redistribute to [8, 3]; thresholds
    EXT = spool.tile([IMGS, 3], dt.float32)
    nc.sync.dma_start(EXT[:, :], EXTR[0:1, :])
    TH = spool.tile([IMGS, 1], dt.float32)   # theta - delta
    nc.vector.tensor_scalar(TH[:], EXT[:, 0:1], -1e-5, None, op0=Alu.add)
    T1 = spool.tile([IMGS, 1], dt.float32)
    nc.vector.tensor_scalar(T1[:], EXT[:, 2:3], 0.65, None, op0=Alu.mult)
    THR = spool.tile([IMGS, 1], dt.float32)
    nc.vector.scalar_tensor_tensor(THR[:], EXT[:, 1:2], 0.35, T1[:], op0=Alu.mult, op1=Alu.add)

    SEL = spool.tile([IMGS, N], dt.float32)
    nc.vector.tensor_scalar(SEL[:], MS[:], TH[:], None, op0=Alu.is_ge)
    HI = spool.tile([IMGS, N], dt.float32)
    nc.vector.tensor_scalar(HI[:], MS[:], THR[:], None, op0=Alu.is_ge)

    PS = sm2.tile([IMGS, N], dt.float32, tag="scr")
    nc.vector.tensor_tensor_scan(PS[:], SEL[:], SEL[:], 0.0, op0=Alu.add, op1=Alu.bypass)
    SPOS = spool.tile([IMGS, N], dt.float32)
    nc.vector.scalar_tensor_tensor(SPOS[:], PS[:], 0.0, SEL[:], op0=Alu.bypass, op1=Alu.mult)
    nc.vector.tensor_scalar(SPOS[:], SPOS[:], -1.0, None, op0=Alu.add)

    SPT, HIT = [], []
    for t in range(NCH):
        pt = PT[t]
        ps = pst.tile([128, 512], dt.float32, tag="tp")
        nc.tensor.transpose(ps[0:pt, 0:IMGS], SPOS[:, t * 128 : t * 128 + pt], ident[0:IMGS, 0:IMGS])
        sb = spool.tile([128, IMGS], dt.float32, tag=f"spt{t}")
        nc.scalar.copy(sb[0:pt, :], ps[0:pt, 0:IMGS])
        SPT.append(sb)
        ps2 = pst.tile([128, 512], dt.float32, tag="tp")
        nc.tensor.transpose(ps2[0:pt, 0:IMGS], HI[:, t * 128 : t * 128 + pt], ident[0:IMGS, 0:IMGS])
        sb2 = spool.tile([128, IMGS], dt.float16, tag=f"hit{t}")
        nc.scalar.copy(sb2[0:pt, :], ps2[0:pt, 0:IMGS])
        HIT.append(sb2)

    # per image: eq matmuls -> benchmark indices + high flags
    IDX16 = []
    SELC = []
    HIC = []
    for img in range(IMGS):
        ps_ixhi = pssmall.tile([2, K], dt.float32, tag="small")
        EQS5 = []
        for c in range(NCH):
            pc = PT[c]
            EQ = eqp.tile([128, K], dt.float16, tag="eq")
            nc.vector.tensor_scalar(
                EQ[0:pc, :], io144[0:pc, :], SPT[c][0:pc, img : img + 1], None,
                op0=Alu.is_equal,
            )
            LH = mscsm.tile([128, 2], dt.float16, tag="lh")
            nc.scalar.copy(LH[0:pc, 0:1], tcol[0:pc, c : c + 1])
            nc.scalar.copy(LH[0:pc, 1:2], HIT[c][0:pc, img : img + 1])
            nc.tensor.matmul(
                ps_ixhi[:, :], LH[0:pc, :], EQ[0:pc, :],
                start=(c == 0), stop=(c == 4),
            )
            EQS5.append(EQ)
        oi = mscsm.tile([1, K], dt.int32, tag="oi")
        nc.vector.tensor_copy(oi[:, :], ps_ixhi[0:1, :])
        nc.sync.dma_start(idx_out[img : img + 1, :], oi[0:1, :])
        HX = mscsm.tile([2, K], dt.float32, tag="hx")
        nc.scalar.copy(HX[:, :], ps_ixhi[0:2, :])
        psb = pst.tile([128, 512], dt.float32, tag="tp")
        nc.tensor.transpose(psb[0:128, 0:2], HX[0:2, 0:128], ident[0:2, 0:2])
        nc.tensor.transpose(psb[0:16, 2:4], HX[0:2, 128:144], ident[0:2, 0:2])
        sc = perim.tile([128, 2], dt.float32, tag="selc")
        nc.scalar.copy(sc[0:128, 0:1], psb[0:128, 0:1])
        nc.scalar.copy(sc[0:16, 1:2], psb[0:16, 2:3])
        hc = perim.tile([128, 2], dt.float32, tag="hic")
        nc.vector.tensor_scalar(hc[0:128, 0:1], psb[0:128, 1:2], -1.0, 1.0, op0=Alu.mult, op1=Alu.add)
        nc.vector.tensor_scalar(hc[0:16, 1:2], psb[0:16, 3:4], -1.0, 1.0, op0=Alu.mult, op1=Alu.add)
        SELC.append(sc)
        HIC.append(hc)
        # second chain: indices in 16-wrap order for dma_gather idx tiles
        ps_perm = pssmall.tile([2, K], dt.float32, tag="small")
        for c in range(NCH):
            pc = PT[c]
            eqpi = EQS5[c][0:pc, :].rearrange("p (s q) -> p q s", s=9, q=16)
            nc.tensor.matmul(
                ps_perm[0:1, :], tcol[0:pc, c : c + 1], eqpi,
                start=(c == 0), stop=(c == 4),
            )
        i16p = mscsm.tile([1, K], dt.int16, tag="i16p")
        nc.vector.tensor_copy(i16p[:, :], ps_perm[0:1, :])
        IDXS = perim.tile([128, 9], dt.int16, tag="idxs")
        for b in range(8):
            nc.sync.dma_start(IDXS[16 * b : 16 * b + 16, :], i16p[0:1, :])
        IDX16.append(IDXS)

    # ---------------- stage H: heavy per-image pipeline ----------------
    for img in range(IMGS):
        hsim = []
        for t in range(NCH):
            pt = PT[t]
            h = hsimp.tile([128, D], dt.float32, tag="hsim")
            nc.sync.dma_start(h[0:pt, :], hs_sim[img, t * 128 : t * 128 + pt, :])
            hsim.append(h)
        ssq = mscsm.tile([128, 5], dt.float32, tag="ssq")
        nc.gpsimd.memset(ssq[:, :], 1.0)
        for t in range(NCH):
            pt = PT[t]
            sq = sqp.tile([128, D], dt.float32, tag="sq")
            nc.scalar.activation(
                sq[0:pt, :], hsim[t][0:pt, :], Act.Square, accum_out=ssq[0:pt, t : t + 1]
            )
        nrm = mscsm.tile([128, 5], dt.float32, tag="nrm")
        nc.scalar.sqrt(nrm[:, :], ssq[:, :])
        inv = mscsm.tile([128, 5], dt.float32, tag="inv")
        nc.vector.reciprocal(inv[:, :], nrm[:, :])
        for t in range(NCH):
            pt = PT[t]
            nc.vector.tensor_scalar_mul(hsim[t][0:pt, :], hsim[t][0:pt, :], inv[0:pt, t : t + 1])

        # transpose normalized rows -> hnT[db] [128(d), 576(j)]
        hnT = []
        for db in range(DCH):
            ht = hntp.tile([128, N], MMDT, tag="hnt")
            ps4 = pst.tile([128, 512], dt.float32, tag="tp")
            for jb in range(4):
                nc.tensor.transpose(
                    ps4[:, jb * 128 : (jb + 1) * 128],
                    hsim[jb][:, db * 128 : (db + 1) * 128],
                    ident[:, :],
                )
            if db % 2 == 0:
                nc.scalar.copy(ht[:, 0:512], ps4[:, 0:512])
            else:
                nc.vector.tensor_copy(ht[:, 0:512], ps4[:, 0:512])
            ps1 = pst.tile([128, 512], dt.float32, tag="tp")
            nc.tensor.transpose(
                ps1[:, 0:64], hsim[4][0:64, db * 128 : (db + 1) * 128], ident[0:64, 0:64]
            )
            if db % 2 == 0:
                nc.scalar.copy(ht[:, 512:576], ps1[:, 0:64])
            else:
                nc.vector.tensor_copy(ht[:, 512:576], ps1[:, 0:64])
            hnT.append(ht)

        # gather raw selected rows, transpose -> selT[db] [128(d), 144(k)]
        hsel = hselp.tile([128, 2 * D], dt.float32, tag="hsel")
        hselv = hsel[:].rearrange("p (c d) -> p c d", c=2, d=D)
        nc.gpsimd.dma_gather(
            hselv, hs_sim[img].flatten_outer_dims(), IDX16[img][:, :],
            num_idxs=K, num_idxs_reg=K, elem_size=D, queue_num=0,
        )
        selT = []
        for db in range(DCH):
            ps = pst.tile([128, 512], dt.float32, tag="tp")
            nc.tensor.transpose(
                ps[:, 0:128], hsel[:, db * 128 : (db + 1) * 128], ident[:, :]
            )
            nc.tensor.transpose(
                ps[:, 128:144],
                hsel[0:16, D + db * 128 : D + (db + 1) * 128],
                ident[0:16, 0:16],
            )
            st = seltp.tile([128, K], MMDT, tag="selt")
            nc.scalar.copy(st[:, :], ps[:, 0:144])
            selT.append(st)

        # gather penalty rows (fp16, padded table)
        peng = penp.tile([128, 2 * 640], dt.float16, tag="peng")
        pengv = peng[:].rearrange("p (c d) -> p c d", c=2, d=640)
        nc.gpsimd.dma_gather(
            pengv, pen16[:].flatten_outer_dims(), IDX16[img][:, :],
            num_idxs=K, num_idxs_reg=K, elem_size=640, queue_num=0,
        )

        hagg = []
        for t in range(NCH):
            pt = PT[t]
            h = haggp.tile([128, D], MMDT, tag="hagg")
            if MM_MODE == "f32r":
                stg = stgp.tile([128, D], dt.float32, tag="stg")
                nc.sync.dma_start(stg[0:pt, :], hs_agg[img, t * 128 : t * 128 + pt, :])
                if t % 2 == 0:
                    nc.vector.tensor_copy(h[0:pt, :], stg[0:pt, :])
                else:
                    nc.scalar.copy(h[0:pt, :], stg[0:pt, :])
            else:
                nc.sync.dma_start(h[0:pt, :].bitcast(dt.float32), hs_agg[img, t * 128 : t * 128 + pt, :])
            hagg.append(h)

        # sim matmuls + epilogue per k-tile; W^T built for the AGG matmul
        WTs = []
        for _wti in range(NCH):
            wt_t = wtp.tile([128, K], MMDT, tag="wt")
            WTs.append(wt_t)
        for kt in range(2):
            pk = KT[kt]
            pssA = pssim.tile([128, 288], dt.float32, tag="sim")
            pssB = pssim.tile([128, 288], dt.float32, tag="sim")
            for db in range(DCH):
                lhs = selT[db][:, kt * 128 : kt * 128 + pk]
                nc.tensor.matmul(
                    pssA[0:pk, :], _mm(lhs), _mm(hnT[db][:, 0:288]),
                    start=(db == 0), stop=(db == 7),
                )
                nc.tensor.matmul(
                    pssB[0:pk, :], _mm(lhs), _mm(hnT[db][:, 288:576]),
                    start=(db == 0), stop=(db == 7),
                )
            reluS = wp.tile([128, N], dt.float32, tag="relu")
            nc.scalar.activation(reluS[0:pk, 0:288], pssA[0:pk, :], Act.Relu)
            nc.scalar.activation(reluS[0:pk, 288:576], pssB[0:pk, :], Act.Relu)
            cpen = wp.tile([128, N], dt.float32, tag="cpen")
            rsum = mscsm.tile([128, 1], dt.float32, tag="rsum")
            nc.vector.tensor_tensor(
                cpen[0:pk, :], reluS[0:pk, :],
                peng[0:pk, kt * 640 : kt * 640 + 576], Alu.mult,
            )
            nc.vector.tensor_reduce(rsum[0:pk, :], cpen[0:pk, :], axis=AX.X, op=Alu.add)
            rq = mscsm.tile([128, 1], dt.float32, tag="rq")
            nc.vector.tensor_scalar(rq[0:pk, :], rsum[0:pk, :], 1e-8, None, op0=Alu.add)
            nc.vector.reciprocal(rq[0:pk, :], rq[0:pk, :])
            W = wp.tile([128, N], dt.float32, tag="wtile")
            nc.vector.tensor_scalar(
                W[0:pk, :], cpen[0:pk, :], rq[0:pk, :], HIC[img][0:pk, kt : kt + 1],
                op0=Alu.mult, op1=Alu.mult,
            )
            EQS = wp.tile([128, N], dt.uint8, tag="eqs")
            nc.vector.tensor_scalar(
                EQS[0:pk, :], io576[0:pk, :], SELC[img][0:pk, kt : kt + 1], None,
                op0=Alu.is_equal,
            )
            nc.vector.copy_predicated(W[0:pk, :], EQS[0:pk, :], ones[0:pk, :])
            for jb in range(NCH):
                pj = PT[jb]
                psw = pst.tile([128, 512], dt.float32, tag="tp")
                nc.tensor.transpose(
                    psw[0:pj, 0:pk], W[0:pk, jb * 128 : jb * 128 + pj],
                    ident[0:pk, 0:pk],
                )
                nc.scalar.copy(WTs[jb][0:pj, kt * 128 : kt * 128 + pk], psw[0:pj, 0:pk])

        # AGG: out[k, dd] = sum_j WT[j, k] * hagg[j, dd]
        for kt in range(2):
            pk = KT[kt]
            ost = msc.tile([128, D], dt.float32, tag="ost")
            for h in range(2):
                pa = psagg.tile([128, 512], dt.float32, tag="agg")
                for jb in range(NCH):
                    pj = PT[jb]
                    nc.tensor.matmul(
                        pa[0:pk, :],
                        _mm(WTs[jb][0:pj, kt * 128 : kt * 128 + pk]),
                        _mm(hagg[jb][0:pj, h * 512 : (h + 1) * 512]),
                        start=(jb == 0), stop=(jb == 4),
                    )
                if h == 0:
                    nc.scalar.copy(ost[0:pk, 0:512], pa[0:pk, :])
                else:
                    nc.vector.tensor_copy(ost[0:pk, 512:1024], pa[0:pk, :])
            nc.sync.dma_start(
                agg_out[img, kt * 128 : kt * 128 + pk, :], ost[0:pk, :]
            )


_CACHE = {}


def _build():
    if "nc" in _CACHE:
        return _CACHE["nc"], _CACHE["names"]
    nc = bacc.Bacc("TRN2", target_bir_lowering=False, debug=False)
    in_specs = [
        ("hs_agg", [IMGS, N, D], dt.float32),
        ("hs_sim", [IMGS, N, D], dt.float32),
        ("metric", [IMGS, N], dt.float32),
        ("pen16", [N, 640], dt.float16),
        ("iota576", [128, N], dt.float32),
        ("iota144", [128, K], dt.float32),
        ("targets", [128, 3], dt.float32),
        ("tcol16", [128, 5], dt.float16),
    ]
    ins = [nc.dram_tensor(n, s, d, kind="ExternalInput").ap() for n, s, d in in_specs]
    outs = [
        nc.dram_tensor("agg", [IMGS, K, D], dt.float32, kind="ExternalOutput").ap(),
        nc.dram_tensor("idx", [IMGS, K], dt.int32, kind="ExternalOutput").ap(),
    ]
    with tile.TileContext(nc) as tc:
        kern(tc, outs, ins)
    nc.compile()
    _CACHE["nc"] = nc
    _CACHE["names"] = [n for n, _, _ in in_specs]
    return nc, _CACHE["names"]


def _run(inputs, trace=False, trace_kwargs=None):
    nc, names = _build()
    consts = host_constants()
    in_maps = []
    for core in range(NCORES):
        sl = slice(core * IMGS, (core + 1) * IMGS)
        m = {
            "hs_agg": np.ascontiguousarray(
                inputs["hidden_states_for_aggregation"][sl]
            ).astype(np.float32, copy=False),
            "hs_sim": np.ascontiguousarray(
                inputs["hidden_states_for_sim"][sl]
            ).astype(np.float32, copy=False),
            "metric": np.ascontiguousarray(inputs["cls_attention_map"][sl]).astype(
                np.float32, copy=False
            ),
        }
        m.update(consts)
        in_maps.append(m)
    res = bass_utils.run_bass_kernel_spmd(
        nc, in_maps, core_ids=list(range(NCORES)), trace=trace,
        trace_kwargs=trace_kwargs or {},
    )
    agg = np.concatenate([res.results[c]["agg"] for c in range(NCORES)], axis=0)
    idx = np.concatenate([res.results[c]["idx"] for c in range(NCORES)], axis=0)
    return (agg, idx.astype(np.int32)), res


def kernel(**inputs):
    (agg, idx), _ = _run(inputs, trace=False)
    return agg, idx


if __name__ == "__main__":
    rng = np.random.default_rng(0)
    ins = {
        "hidden_states_for_aggregation": rng.standard_normal((B, N, D), dtype=np.float32),
        "hidden_states_for_sim": rng.standard_normal((B, N, D), dtype=np.float32),
        "cls_attention_map": rng.random((B, N), dtype=np.float32),
    }
    out, idx = kernel(**ins)
    print(out.shape, idx.shape, idx.dtype)
